# revision 4
# baseline (speedup 1.0000x reference)
"""MoE routing kernel (nn_EnhancedBrain) for Trainium2, 8 NeuronCores.

Strategy
--------
The router (mean-pool -> tiny MLP -> softmax -> top-3 -> renormalize) costs
~8 MFLOP vs ~1.7 TFLOP for the expert MLPs, and its only effect on the math
is which 3 of the 8 zone weights are nonzero per batch row.  It runs on host
in float64; the device computes exactly the nonzero-weight expert MLPs.

Sharding: by batch row.  Core c gets rows 2c and 2c+1, each with its 3
selected experts -> 6 equal expert-MLP passes per core, perfectly balanced,
no collectives.  The renormalized top-k weight is folded into that pair's Wb
copy on host.

Precision: fp8 e4m3 with DoubleRow matmuls (2 fp8 MACs/PE-cell/cycle, ~2x
the bf16/fp32r row rate).  Host quantizes x (*16), Wa (*64) and Wb
(*top_w*256) to e4m3 in the DoubleRow bank-interleaved layout
[..., 128, 2, ...]; the 1/(16*64) descale folds into the gelu activation
scale and the residual-add path carries z in units of z*256 (exact powers of
two), undone by a final scaled copy.  PSUM accumulation stays fp32, the
residual path stays fp32, so end-to-end rel err ~1.9e-2 < 2e-2.

Device kernel (per core), per batch row and token half (T=1024, SBUF fit):
    z^T[d, t]  = 256 * x^T[d, t]                  (fp32, ACT copy)
    for each of the 3 experts, over f-chunks of 1024:
      h^T[f, t] = gelu_tanh((Wa'^T x'^T) / 1024)  (PE fp8-DR + ACT -> fp8)
      z^T      += Wb''^T h^T                      (PE fp8-DR chain; DVE add)
    y^T[d, t]  = z^T / 256                        (ACT copy, exact scale)
Everything stays in the transposed [feature, token] layout so neither layer
needs a transpose; the host hands x^T in and transposes y^T back on gather.
"""

import ml_dtypes
import numpy as np

import concourse.bass as bass
import concourse.mybir as mybir
import concourse.tile as tile
from concourse import bacc
from concourse.bass_utils import run_bass_kernel_spmd

B, S, D, F = 16, 2048, 1024, 4096
NZONES, TOPK = 8, 3
NCORES = 8
NB = B // NCORES            # batch rows per core = 2
NP = NB * TOPK              # (row, expert) pairs per core = 6
TH = 2                      # token halves per row
T = S // TH                 # tokens per half = 1024
TB = 512                    # matmul moving-dim block (psum bank = 512 fp32)
FC = 1024                   # f-chunk held in SBUF at once
GROUP = 2                   # f-chunks per L2 psum chain
P = 128
NDC = D // (2 * P)          # DoubleRow 256-deep d blocks = 4
NFB = F // (2 * P)          # DoubleRow 256-deep f blocks = 16
NHB = FC // (2 * P)         # 256-deep f blocks per chunk = 4
NTB = T // TB               # token blocks per half = 2
NZ = D // P                 # z tiles = 8

SX, SA, S2 = 16.0, 64.0, 256.0   # exact powers of two

F32 = mybir.dt.float32
F8 = mybir.dt.float8e4
E4NP = ml_dtypes.float8_e4m3     # == mybir.dt.np(float8e4); max normal 240
DR = mybir.MatmulPerfMode.DoubleRow
GELU = mybir.ActivationFunctionType.Gelu_apprx_tanh
COPY = mybir.ActivationFunctionType.Copy

_compiled_nc = None


def _build_nc(reps=1):
    from contextlib import nullcontext

    nc = bacc.Bacc("TRN2", target_bir_lowering=False)
    xt32 = nc.dram_tensor("xt32", [NB, D, S], F32, kind="ExternalInput")
    xt8 = nc.dram_tensor("xt8", [NB, NDC, P, 2, S], F8, kind="ExternalInput")
    wa8 = nc.dram_tensor("wa8", [NP, NDC, P, 2, F], F8, kind="ExternalInput")
    wb8 = nc.dram_tensor("wb8", [NP, NFB, P, 2, D], F8, kind="ExternalInput")
    y = nc.dram_tensor("y", [NB, D, S], F32, kind="ExternalOutput")

    nfc = F // FC

    with tile.TileContext(nc) as tc:
        with (
            tc.tile_pool(name="x8p", bufs=2 * NDC + 1) as x8_pool,
            tc.tile_pool(name="x32p", bufs=3) as x32_pool,
            tc.tile_pool(name="zp", bufs=NZ + 2) as z_pool,
            tc.tile_pool(name="yp", bufs=3) as y_pool,
            tc.tile_pool(name="wap", bufs=2 * NDC + 2) as wa_pool,
            tc.tile_pool(name="wbp", bufs=4 * NHB) as wb_pool,
            tc.tile_pool(name="hp", bufs=4 * NHB) as h_pool,
            tc.tile_pool(name="ps1", bufs=4, space="PSUM") as ps1,
            tc.tile_pool(name="ps2", bufs=4, space="PSUM") as ps2,
            tc.For_i(0, reps, 1) if reps > 1 else nullcontext(),
        ):
            for bi in range(NB):
                for hh in range(TH):
                    t0 = hh * T
                    # x moving operand, DoubleRow layout [128, 2, T] per
                    # 256-deep d block
                    x8s = []
                    for dc in range(NDC):
                        xtile = x8_pool.tile([P, 2, T], F8, tag="x8")
                        nc.sync.dma_start(xtile[:], xt8[bi, dc, :, :, t0:t0 + T])
                        x8s.append(xtile)
                    # residual init: z = x * S2 (fp32, exact)
                    zts = []
                    for dz in range(NZ):
                        x32 = x32_pool.tile([P, T], F32, tag="x32")
                        nc.sync.dma_start(
                            x32[:], xt32[bi, dz * P:(dz + 1) * P, t0:t0 + T]
                        )
                        ztile = z_pool.tile([P, T], F32, tag="z")
                        nc.scalar.activation(ztile[:], x32[:], COPY, scale=S2)
                        zts.append(ztile)

                    def emit_l2(chunks):
                        # z^T += (Wb*w*S2)^T h^T for a group of f-chunks in
                        # one PSUM chain (the accumulation is linear so
                        # grouping across chunk/expert boundaries is exact)
                        hts = [t for c in chunks for t in c[0]]
                        wbts = [t for c in chunks for t in c[1]]
                        nch = len(hts)
                        for dz in range(NZ):
                            for tb in range(NTB):
                                s = slice(tb * TB, (tb + 1) * TB)
                                pz = ps2.tile([P, TB], F32, tag="ps2")
                                for fi in range(nch):
                                    nc.tensor.matmul(
                                        pz[:],
                                        wbts[fi][:, :, dz * P:(dz + 1) * P],
                                        hts[fi][:, :, s],
                                        start=(fi == 0),
                                        stop=(fi == nch - 1),
                                        perf_mode=DR,
                                    )
                                nc.vector.tensor_tensor(
                                    zts[dz][:, s],
                                    zts[dz][:, s],
                                    pz[:],
                                    op=mybir.AluOpType.add,
                                )

                    # Software-pipelined with skew 1: L1 of chunk c+1 is
                    # emitted before L2 of chunk c so the in-order PE covers
                    # the gelu tail of chunk c with chunk c+1's matmuls.
                    pending = []
                    for k in range(TOPK):
                        pr = bi * TOPK + k
                        for fc in range(nfc):
                            f0 = fc * FC
                            wats = []
                            for dc in range(NDC):
                                wt = wa_pool.tile([P, 2, FC], F8, tag="wa")
                                nc.sync.dma_start(
                                    wt[:], wa8[pr, dc, :, :, f0:f0 + FC]
                                )
                                wats.append(wt)
                            hts = []
                            for hb in range(NHB):
                                ht = h_pool.tile([P, 2, T], F8, tag="h")
                                for i in range(2):
                                    foff = (hb * 2 + i) * P
                                    for tb in range(NTB):
                                        s = slice(tb * TB, (tb + 1) * TB)
                                        ph = ps1.tile([P, TB], F32, tag="ps1")
                                        for dc in range(NDC):
                                            nc.tensor.matmul(
                                                ph[:],
                                                wats[dc][:, :, foff:foff + P],
                                                x8s[dc][:, :, s],
                                                start=(dc == 0),
                                                stop=(dc == NDC - 1),
                                                perf_mode=DR,
                                            )
                                        nc.scalar.activation(
                                            ht[:, i, s], ph[:], GELU,
                                            scale=1.0 / (SX * SA),
                                        )
                                hts.append(ht)
                            wbts = []
                            for hb in range(NHB):
                                wt = wb_pool.tile([P, 2, D], F8, tag="wb")
                                nc.sync.dma_start(
                                    wt[:],
                                    wb8[pr, fc * NHB + hb, :, :, :],
                                )
                                wbts.append(wt)
                            pending.append((hts, wbts))
                            if len(pending) == GROUP + 1:
                                emit_l2(pending[:GROUP])
                                pending = pending[GROUP:]
                    emit_l2(pending)
                    # y = z / S2 (exact scale), then store
                    for dz in range(NZ):
                        yt = y_pool.tile([P, T], F32, tag="y")
                        nc.scalar.activation(
                            yt[:], zts[dz][:], COPY, scale=1.0 / S2
                        )
                        nc.sync.dma_start(
                            y[bi, dz * P:(dz + 1) * P, t0:t0 + T], yt[:]
                        )
    nc.compile()
    return nc


def _route(x, W1, b1, W2, b2):
    """Host router in float64; reproduces jax.lax.top_k tie-breaking."""
    pooled = x.mean(axis=1, dtype=np.float64)
    h = np.tanh(pooled @ W1.astype(np.float64) + b1.astype(np.float64))
    logits = h @ W2.astype(np.float64) + b2.astype(np.float64)
    e = np.exp(logits - logits.max(axis=-1, keepdims=True))
    probs = e / e.sum(axis=-1, keepdims=True)
    top_i = np.argsort(-probs, axis=-1, kind="stable")[:, :TOPK]
    top_p = np.take_along_axis(probs, top_i, axis=-1)
    top_w = top_p / top_p.sum(axis=-1, keepdims=True)
    return top_i, top_w


def _q8(a, s):
    """e4m3 quantize with scale; clip to the TRN E4M3 max normal +-240."""
    return np.clip(a * np.float32(s), -240.0, 240.0).astype(E4NP)


def _dr_rows(a2d, nblk):
    """[R, C] -> [R/256 blocks, 128, 2, C] DoubleRow bank interleave:
    out[b, p, i, c] = a2d[b*256 + i*128 + p, c]."""
    r, c = a2d.shape
    return np.ascontiguousarray(
        a2d.reshape(nblk, 2, P, c).transpose(0, 2, 1, 3)
    )


def _prepare_in_maps(inputs):
    x = np.ascontiguousarray(np.asarray(inputs["x"], dtype=np.float32))
    Wa = np.asarray(inputs["Wa"], dtype=np.float32)
    Wb = np.asarray(inputs["Wb"], dtype=np.float32)

    top_i, top_w = _route(
        x,
        np.asarray(inputs["W1"]),
        np.asarray(inputs["b1"]),
        np.asarray(inputs["W2"]),
        np.asarray(inputs["b2"]),
    )

    wa8_by_e = [_dr_rows(_q8(Wa[e], SA), NDC) for e in range(NZONES)]

    in_maps = []
    for c in range(NCORES):
        rows = [NB * c + i for i in range(NB)]
        xt32 = np.ascontiguousarray(x[rows].transpose(0, 2, 1))
        xt8 = np.stack([_dr_rows(_q8(xt32[i], SX), NDC) for i in range(NB)])
        wa_l, wb_l = [], []
        for i, b in enumerate(rows):
            for k in range(TOPK):
                e = int(top_i[b, k])
                wa_l.append(wa8_by_e[e])
                wb_l.append(_dr_rows(_q8(Wb[e], top_w[b, k] * S2), NFB))
        in_maps.append({
            "xt32": xt32,
            "xt8": np.ascontiguousarray(xt8),
            "wa8": np.ascontiguousarray(np.stack(wa_l)),
            "wb8": np.ascontiguousarray(np.stack(wb_l)),
        })
    return in_maps


def kernel(x, W1, b1, W2, b2, Wa, Wb):
    global _compiled_nc
    if _compiled_nc is None:
        _compiled_nc = _build_nc()
    nc = _compiled_nc

    in_maps = _prepare_in_maps(
        {"x": x, "W1": W1, "b1": b1, "W2": W2, "b2": b2, "Wa": Wa, "Wb": Wb}
    )

    res = run_bass_kernel_spmd(nc, in_maps, core_ids=list(range(NCORES)))

    y = np.empty((B, S, D), dtype=np.float32)
    for c in range(NCORES):
        yt = res.results[c]["y"]                      # [NB, D, S]
        for i in range(NB):
            y[NB * c + i] = yt[i].T
    return y
